# revision 21
# baseline (speedup 1.0000x reference)
"""MoE MLP (top-2 routing, 8 experts) on 8 Trainium2 NeuronCores.

Load-balanced expert-parallel design. The router runs on the host and
doubles as the dispatch step. Instead of one expert per core (which
pads every core to the *largest* expert's token count, 2176), each
core processes exactly C=2056 token-columns in 5 fixed-size chunks
(e.g. [512, 368, 512, 336, 328]); each chunk has its OWN weight
tensors that the host binds to whichever expert that chunk serves on
that core. A small exact-cover solve assigns experts to the 40
(core, chunk) slots so every token-expert pair lands in exactly one
slot. The SPMD instruction stream is identical on every core; only
the bound data differs. All chunk sizes are >= ~280 tokens so no
matmul is LDWEIGHTS-bound.

Each chunk runs the fused gelu-MLP in a transposed layout (tokens on
the free axis):

    yT = w * (W_out^T @ gelu(W_in^T @ xT + b_in) + b_out)

Phase A streams W_in in 512-wide F-stripes; phase B streams W_out in
[512-f x 512-d] tiles, accumulating 4 d-blocks (one PSUM bank each)
per 512-wide d-half. Weights are never SBUF-resident, so chunks are
free to use different experts. To keep DMA descriptor lines fat (8KB
and 4KB per partition instead of 1KB), the host pre-packs W_in, W_out
and x into partition-major layouts. Matmuls run in fp16 (4x fp32 PE
throughput, ~4e-4 end-to-end error vs the fp32 reference).

The host scatter-adds the per-slot results back into [B,S,D].
"""

import contextlib
import ctypes
import os
import sys
import types
from contextlib import ExitStack

import numpy as np

import concourse.bass as bass
import concourse.mybir as mybir
import concourse.tile as tile
from concourse import bacc
from concourse.bass_utils import run_bass_kernel_spmd


def _install_ntff_hook():
    """Provide antenv.axon_hooks (absent in this image) so BASS_TRACE=1
    can capture NTFF profiles through the axon PJRT .so. No-op if the
    module already exists or the .so/symbols are unavailable."""
    try:
        from antenv.axon_hooks import get_axon_ntff_profile_hook  # noqa: F401
        return
    except ImportError:
        pass
    so_path = "/opt/axon/libaxon_pjrt.so"
    if not os.path.exists(so_path):
        return
    try:
        lib = ctypes.CDLL(so_path)
    except OSError:
        return
    if not hasattr(lib, "axon_start_nrt_profile"):
        return
    lib.axon_start_nrt_profile.argtypes = [
        ctypes.POINTER(ctypes.c_int64), ctypes.c_size_t]
    lib.axon_start_nrt_profile.restype = ctypes.c_int64
    lib.axon_stop_nrt_profile.argtypes = [ctypes.c_char_p]
    lib.axon_stop_nrt_profile.restype = ctypes.c_int64

    @contextlib.contextmanager
    def _hook(output_dir, device_ids):
        import jax
        jax.devices()  # force PJRT init so the .so's client exists
        if device_ids:
            ids = (ctypes.c_int64 * len(device_ids))(*device_ids)
            rc = lib.axon_start_nrt_profile(ids, len(device_ids))
        else:
            rc = lib.axon_start_nrt_profile(None, 0)
        if rc != 0:
            raise RuntimeError(f"axon_start_nrt_profile rc={rc}")
        try:
            yield
        finally:
            n = lib.axon_stop_nrt_profile(str(output_dir).encode())
            print(f"ntff profile: {n} file(s) -> {output_dir}", file=sys.stderr)

    import antenv
    mod = types.ModuleType("antenv.axon_hooks")
    mod.get_axon_ntff_profile_hook = lambda: _hook
    mod.set_axon_ntff_profile_hook = lambda h: None
    sys.modules["antenv.axon_hooks"] = mod
    antenv.axon_hooks = mod


B, S, D, F, E = 4, 2048, 1024, 4096, 8
T = B * S
TOP_K = 2
NCORES = 8
P = 128
ND, NF = D // P, F // P  # 8, 32
NS = 8                   # 512-wide F-stripes in phase A
NCHUNK = 5
WARMUP_MM = 150

# test.py pokes these for profiling info
LAST_RESULT = None

_cache = {}


def _solve_slots(counts):
    """Choose per-core chunk sizes (two 512s + three in [288, 512]) and
    assign experts to the 40 (core, chunk) slots. Per-expert slot
    multisets must cover its token count; column budgets are 8 slots
    per chunk position. Returns (sizes5, sol) where sizes5 is the
    per-position size list (position order = processing order) and
    sol[e] maps size -> slot count."""
    import itertools

    def solve(colsizes, budgets):
        opts = []
        for cnt in counts:
            o = []
            for combo in itertools.product(*[range(b + 1) for b in budgets]):
                cap = sum(s * k for s, k in zip(colsizes, combo))
                if cap >= cnt and cap - cnt < 512:
                    o.append(combo)
            opts.append(o)
        states = {tuple([0] * len(budgets)): []}
        for o in opts:
            ns = {}
            for st, hist in states.items():
                for combo in o:
                    k = tuple(a + b for a, b in zip(st, combo))
                    if all(x <= b for x, b in zip(k, budgets)) and k not in ns:
                        ns[k] = hist + [combo]
            states = ns
        return states.get(tuple(budgets))

    for C in range(2056, 2600, 8):
        rem = C - 1024
        for z5 in range(512, 287, -8):
            for z4 in range(512, z5 - 1, -8):
                z3 = rem - z4 - z5
                if not (z4 <= z3 <= 512):
                    continue
                sizes = {}
                for s, b in [(512, 16), (z3, 8), (z4, 8), (z5, 8)]:
                    sizes[s] = sizes.get(s, 0) + b
                cs = sorted(sizes, reverse=True)
                bs = [sizes[s] for s in cs]
                sol = solve(cs, bs)
                if sol:
                    sizes5 = [512, z4, 512, z3, z5]  # processing order
                    soldicts = [dict(zip(cs, combo)) for combo in sol]
                    return sizes5, soldicts
    raise RuntimeError(f"no slot assignment for counts={counts}")


def _build_grid(sizes5, sol):
    """grid[core][pos] = expert id. Chop each size-class's slot list
    (expert ids with multiplicity) across the positions of that size."""
    bysize = {}
    for e, d in enumerate(sol):
        for s, k in d.items():
            bysize.setdefault(s, []).extend([e] * k)
    taken = {s: 0 for s in bysize}
    grid = [[None] * NCHUNK for _ in range(NCORES)]
    for pos in range(NCHUNK):
        s = sizes5[pos]
        lst = bysize[s]
        for core in range(NCORES):
            grid[core][pos] = lst[taken[s]]
            taken[s] += 1
    return grid


def _build_bass(sizes):
    dt = mybir.dt
    io_dt = dt.float16
    C = sum(sizes)
    offs = np.cumsum([0] + list(sizes))[:NCHUNK]
    nc = bacc.Bacc("TRN2", target_bir_lowering=False, debug=False)

    # Packed layouts (host pre-shuffled, partition-major, fat DMA lines):
    #  xp   [P, 8*C]        x[dn*128+p, c]            -> [p, chunk: dn, c]
    #  win  [P, 8*8*512]    W_in[dn*128+p, fo*512+f]  -> [p, fo, dn, f]
    #  wout [P, 2*8*4*512]  W_out[fn*128+p, dh*512+d] -> [p, dh, g, fc4, d]
    #  yp   [P, 8*C] out                              <- [p, chunk: dn, c]
    xp = nc.dram_tensor("xp", [P, ND * C], io_dt, kind="ExternalInput")
    wins = [nc.dram_tensor(f"win{j}", [P, NS * ND * 512], io_dt,
                           kind="ExternalInput") for j in range(NCHUNK)]
    wouts = [nc.dram_tensor(f"wout{j}", [P, 2 * NS * 4 * 512], io_dt,
                            kind="ExternalInput") for j in range(NCHUNK)]
    bins = [nc.dram_tensor(f"bin{j}", [F], dt.float32, kind="ExternalInput")
            for j in range(NCHUNK)]
    bouts = [nc.dram_tensor(f"bout{j}", [D], dt.float32, kind="ExternalInput")
             for j in range(NCHUNK)]
    wcomb = nc.dram_tensor("wcomb", [P, C], dt.float32, kind="ExternalInput")
    yp = nc.dram_tensor("yp", [P, ND * C], io_dt, kind="ExternalOutput")

    SW = ND * 512  # elements per win stripe per partition

    with tile.TileContext(nc) as tc, ExitStack() as ctx:
        consts = ctx.enter_context(tc.tile_pool(name="consts", bufs=1))
        xpool = ctx.enter_context(tc.tile_pool(name="x", bufs=2))
        winpool = ctx.enter_context(tc.tile_pool(name="win", bufs=8))
        woutpool = ctx.enter_context(tc.tile_pool(name="wout", bufs=8))
        hpool = ctx.enter_context(tc.tile_pool(name="h", bufs=1))
        ypool = ctx.enter_context(tc.tile_pool(name="y", bufs=4))
        wcpool = ctx.enter_context(tc.tile_pool(name="wc", bufs=2))
        psum_h = ctx.enter_context(tc.tile_pool(name="ph", bufs=4, space="PSUM"))
        psum_y = ctx.enter_context(tc.tile_pool(name="py", bufs=1, space="PSUM"))

        # PE warm-up source: memset FIRST on gpsimd (compute op; the
        # SWDGE dma-init cost lands on the later bias loads).
        wu_t = consts.tile([P, P], io_dt)
        nc.gpsimd.memset(wu_t[:], 0.0)

        bin_ts, bout_ts = [], []
        for j in range(NCHUNK):
            bin_t = consts.tile([P, NF], dt.float32, name=f"bin_t{j}",
                                tag=f"bin{j}")
            nc.gpsimd.dma_start(
                bin_t[:], bins[j].ap().rearrange("(fo fi) -> fi fo", fi=P))
            bout_t = consts.tile([P, ND], dt.float32, name=f"bout_t{j}",
                                 tag=f"bout{j}")
            nc.gpsimd.dma_start(
                bout_t[:], bouts[j].ap().rearrange("(do di) -> di do", di=P))
            bin_ts.append(bin_t)
            bout_ts.append(bout_t)

        # Critical path to the first matmul: chunk-0's x and W_in
        # stripes 0-1 land as six ~0.5MB dn-half transfers, three per
        # HWDGE queue. Few fat transfers beat many small pieces in the
        # early window where ring/descriptor overhead dominates.
        ck0 = sizes[0]
        x0_t = xpool.tile([P, ND, ck0], io_dt, tag="x")

        def xhalf(q, lo):
            q.dma_start(
                x0_t[:, lo:lo + 4, :],
                xp.ap()[:, lo * ck0:(lo + 4) * ck0]
                .rearrange("p (dn c) -> p dn c", dn=4))

        xhalf(nc.sync, 0)
        xhalf(nc.scalar, 4)

        # PE HAM warm-up: junk matmuls bridge engine start (~5.5us) to
        # the first real matmul so the PE is warming early.
        wu_p = psum_y.tile([P, 64], dt.float32, name="wu_p", tag="py0")
        for _ in range(WARMUP_MM):
            nc.tensor.matmul(wu_p[:], wu_t[:], wu_t[:, :64],
                             start=True, stop=True)

        x_t = x0_t
        x_next = None
        for ci in range(NCHUNK):
            ck = sizes[ci]
            off = int(offs[ci])
            if ci > 0:
                x_t = x_next
            w_t = wcpool.tile([P, ck], dt.float32, tag="wc")
            nc.gpsimd.dma_start(w_t[:], wcomb.ap()[:, off:off + ck])

            h_t = hpool.tile([P, NF, ck], io_dt, tag="h")

            def a_stripe(fo, win_t=None, ci=ci, ck=ck, x_t=x_t, h_t=h_t):
                """phase A stripe: h[4fo..4fo+3] = gelu(W_in^T x + b)"""
                if win_t is None:
                    win_t = winpool.tile([P, ND, 512], io_dt, tag="win")
                    o = fo * SW
                    nc.sync.dma_start(
                        win_t[:, :4, :],
                        wins[ci].ap()[:, o:o + SW // 2]
                        .rearrange("p (dn f) -> p dn f", dn=4))
                    nc.scalar.dma_start(
                        win_t[:, 4:, :],
                        wins[ci].ap()[:, o + SW // 2:o + SW]
                        .rearrange("p (dn f) -> p dn f", dn=4))
                for j in range(4):
                    fc = fo * 4 + j
                    ph = psum_h.tile([P, ck], dt.float32, tag="ph")
                    for dn in range(ND):
                        nc.tensor.matmul(
                            ph[:],
                            win_t[:, dn, j * P:(j + 1) * P],
                            x_t[:, dn, :],
                            start=(dn == 0),
                            stop=(dn == ND - 1),
                        )
                    nc.scalar.activation(
                        h_t[:, fc, :], ph[:],
                        mybir.ActivationFunctionType.Gelu,
                        bias=bin_ts[ci][:, fc:fc + 1],
                    )

            def b_group(dh, g, pys, ci=ci, h_t=h_t):
                """phase B: accumulate W_out tile g (4 fc) into 4 banks"""
                wt = woutpool.tile([P, 4, 512], io_dt, tag="wout")
                o = (dh * NS + g) * 4 * 512
                eng = nc.sync if g % 2 == 0 else nc.scalar
                eng.dma_start(
                    wt[:], wouts[ci].ap()[:, o:o + 4 * 512]
                    .rearrange("p (f4 d) -> p f4 d", f4=4))
                for j in range(4):
                    fc = g * 4 + j
                    for i in range(4):
                        nc.tensor.matmul(
                            pys[i][:],
                            wt[:, j, i * P:(i + 1) * P],
                            h_t[:, fc, :],
                            start=(fc == 0),
                            stop=(fc == NF - 1),
                        )

            def b_drain(dh, pys, ci=ci, ck=ck, off=off, w_t=w_t):
                for i in range(4):
                    dn = dh * 4 + i
                    y_t = ypool.tile([P, ck], io_dt, tag="y")
                    # one DVE op: (psum + b_out) * w
                    nc.vector.scalar_tensor_tensor(
                        y_t[:], pys[i][:], bout_ts[ci][:, dn:dn + 1], w_t[:],
                        op0=mybir.AluOpType.add, op1=mybir.AluOpType.mult,
                    )
                    eng = nc.sync if i % 2 == 0 else nc.scalar
                    eng.dma_start(
                        yp.ap()[:, ND * off + dn * ck:ND * off + (dn + 1) * ck],
                        y_t[:])

            def new_pys(ck=ck):
                return [psum_y.tile([P, ck], dt.float32, name=f"py_{i}",
                                    tag=f"py{i}") for i in range(4)]

            def x_prefetch(ci=ci):
                nonlocal x_next
                if ci + 1 < NCHUNK:
                    nck = sizes[ci + 1]
                    noff = int(offs[ci + 1])
                    x_next = xpool.tile([P, ND, nck], io_dt, tag="x")
                    eng = nc.sync if ci % 2 == 0 else nc.scalar
                    eng.dma_start(
                        x_next[:],
                        xp.ap()[:, ND * noff:ND * (noff + nck)]
                        .rearrange("p (dn c) -> p dn c", dn=ND))

            # ---- phase A then phase B (streams prefetch under the
            # previous chunk's B window; chunk 0 rides the DMA ramp
            # with 256-wide single-transfer stripes)
            if ci == 0:
                for so in range(16):
                    win_t = winpool.tile([P, ND, 256], io_dt, tag="win")
                    o = so * (ND * 256)
                    eng = nc.sync if so % 2 == 0 else nc.scalar
                    eng.dma_start(
                        win_t[:],
                        wins[0].ap()[:, o:o + ND * 256]
                        .rearrange("p (dn f) -> p dn f", dn=ND))
                    for j in range(2):
                        fc = so * 2 + j
                        ph = psum_h.tile([P, ck], dt.float32, tag="ph")
                        for dn in range(ND):
                            nc.tensor.matmul(
                                ph[:],
                                win_t[:, dn, j * P:(j + 1) * P],
                                x_t[:, dn, :],
                                start=(dn == 0),
                                stop=(dn == ND - 1),
                            )
                        nc.scalar.activation(
                            h_t[:, fc, :], ph[:],
                            mybir.ActivationFunctionType.Gelu,
                            bias=bin_ts[ci][:, fc:fc + 1],
                        )
            else:
                for fo in range(NS):
                    a_stripe(fo)
            x_prefetch()
            for dh in range(2):
                pys = new_pys()
                for g in range(NS):
                    b_group(dh, g, pys)
                b_drain(dh, pys)

    nc.compile()
    return nc


def _get_nc(sizes):
    key = tuple(sizes)
    if key not in _cache:
        _cache[key] = _build_bass(sizes)
    return _cache[key]


def _route(x, W_router):
    """Host-side router: top-2 selection + renormalized weights (fp64).

    Matches jax.lax.top_k on softmax(logits): softmax is monotone so
    top-2 of logits is identical, with ties broken toward lower index
    (argsort stable on -logits).
    """
    lg = x.astype(np.float64) @ W_router.T.astype(np.float64)
    top2 = np.argsort(-lg, axis=1, kind="stable")[:, :TOP_K]
    l1 = np.take_along_axis(lg, top2[:, 0:1], 1)
    l2 = np.take_along_axis(lg, top2[:, 1:2], 1)
    e2 = np.exp(l2 - l1)
    w1 = (1.0 / (1.0 + e2)).astype(np.float32)
    w2 = (e2 / (1.0 + e2)).astype(np.float32)
    return top2, np.concatenate([w1, w2], axis=1)


def _pack_win(w):
    """[D, F] fp32 -> [P, 8*8*512] fp16: [p, fo, dn, f]"""
    return np.ascontiguousarray(
        w.reshape(ND, P, NS, 512).transpose(1, 2, 0, 3)
        .reshape(P, NS * ND * 512), dtype=np.float16)


def _pack_win256(w):
    """[D, F] fp32 -> [P, 16*8*256] fp16: [p, s, dn, f] (256-wide
    stripes; chunk 0 streams these so each stripe is a single 0.5MB
    transfer and the DMA-ramp arrival quantum halves)"""
    return np.ascontiguousarray(
        w.reshape(ND, P, 16, 256).transpose(1, 2, 0, 3)
        .reshape(P, 16 * ND * 256), dtype=np.float16)


def _pack_wout(w):
    """[F, D] fp32 -> [P, 2*8*4*512] fp16: [p, dh, g, fc4, d]"""
    return np.ascontiguousarray(
        w.reshape(NS, 4, P, 2, 512).transpose(2, 3, 0, 1, 4)
        .reshape(P, 2 * NS * 4 * 512), dtype=np.float16)


def kernel(residual, W_router, W_in, b_in, W_out, b_out):
    global LAST_RESULT

    x = np.ascontiguousarray(np.asarray(residual, dtype=np.float32).reshape(T, D))
    W_in = np.asarray(W_in, dtype=np.float32)
    W_out = np.asarray(W_out, dtype=np.float32)
    b_in = np.asarray(b_in, dtype=np.float32)
    b_out = np.asarray(b_out, dtype=np.float32)

    top2, wts = _route(x, np.asarray(W_router, dtype=np.float32))

    idxs, ws = [], []
    for e in range(E):
        sel0 = top2[:, 0] == e
        sel1 = top2[:, 1] == e
        idx = np.concatenate([np.where(sel0)[0], np.where(sel1)[0]])
        w = np.concatenate([wts[sel0, 0], wts[sel1, 1]])
        idxs.append(idx)
        ws.append(w)

    counts = [len(i) for i in idxs]
    sizes, sol = _solve_slots(counts)
    grid = _build_grid(sizes, sol)
    offs = np.cumsum([0] + list(sizes))[:NCHUNK]
    C = sum(sizes)
    nc = _get_nc(sizes)

    # chop each expert's token list across its (core, chunk) slots
    ptr = [0] * E
    fills = {}
    for core in range(NCORES):
        for j in range(NCHUNK):
            e = grid[core][j]
            take = min(sizes[j], counts[e] - ptr[e])
            fills[(core, j)] = (e, idxs[e][ptr[e]:ptr[e] + take],
                                ws[e][ptr[e]:ptr[e] + take])
            ptr[e] += take
    assert ptr == counts, (ptr, counts)

    win_p = [_pack_win(W_in[e]) for e in range(E)]
    win_p256 = [_pack_win256(W_in[e]) for e in range(E)]
    wout_p = [_pack_wout(W_out[e]) for e in range(E)]

    xt = np.ascontiguousarray(x.T.astype(np.float16))  # [D, T]
    in_maps = []
    for core in range(NCORES):
        xp_c = np.zeros((P, ND * C), dtype=np.float16)
        wc_c = np.zeros((P, C), dtype=np.float32)
        m = {"xp": xp_c, "wcomb": wc_c}
        for j in range(NCHUNK):
            e, ids, w = fills[(core, j)]
            o = int(offs[j])
            ckj = sizes[j]
            if len(ids):
                blk = xt[:, ids]  # [D, n]
                pb = np.zeros((P, ND, ckj), dtype=np.float16)
                pb[:, :, :len(ids)] = blk.reshape(ND, P, len(ids)).transpose(1, 0, 2)
                xp_c[:, ND * o:ND * (o + ckj)] = pb.reshape(P, ND * ckj)
            wc_c[:, o:o + len(ids)] = w[None, :]
            m[f"win{j}"] = win_p256[e] if j == 0 else win_p[e]
            m[f"wout{j}"] = wout_p[e]
            m[f"bin{j}"] = b_in[e]
            m[f"bout{j}"] = b_out[e]
        in_maps.append(m)

    if os.environ.get("BASS_TRACE"):
        _install_ntff_hook()
    LAST_RESULT = run_bass_kernel_spmd(nc, in_maps, list(range(NCORES)))

    y = np.zeros((T, D), dtype=np.float32)
    for core in range(NCORES):
        yp_c = LAST_RESULT.results[core]["yp"].astype(np.float32)  # [P, 8*C]
        for j in range(NCHUNK):
            _, ids, _ = fills[(core, j)]
            o = int(offs[j])
            ckj = sizes[j]
            if len(ids):
                blk = yp_c[:, ND * o:ND * (o + ckj)].reshape(P, ND, ckj)
                # blk[p, dn, c] = y[dn*128+p, token c]
                y[ids] += blk.transpose(1, 0, 2).reshape(D, ckj)[:, :len(ids)].T
    return y.reshape(B, S, D)
